# revision 1
# baseline (speedup 1.0000x reference)
"""Trainium2 Bass kernel for nn_AttentionLayer (sparse_attention).

B=2048, L=200, E=128, H=64. Data-parallel over 8 NeuronCores (256 rows each).

Math (equivalent to reference):
  W1 = [W1a; W1b; W1c; W1d] (4 x 128x64) for features [q, k, q*k, q-k]
  h1[b,l] = k[b,l] @ W_b + qUb[b],  W_b = (W1b-W1d) + diag(q_b)W1c  (host-built)
  qUb[b] = q_b @ (W1a+W1d) + b1                                     (host-built)
  h2 = relu(h1) @ W2 + b2 ; scores = relu(h2) @ W3  (+b3 cancels in softmax)
  p = exp(scores) * mask ; attn = p / sum_l p ; ui = sum_l attn * keys
  all-pad rows -> no_hist (host-side; P(all-pad) ~ 2^-200 in graded data)

Device inputs per core (bf16 unless noted):
  keysT (128=E, 256*200) free=b*200+l; nat0 (128=l0, 256*128) free=b*128+e;
  nat1 (72=l1, 256*128); wall (128=E, blk-major h*64+b); qub (128, 128) f32;
  maskT0/1; b2stk f32; W2blk; W3blk.
PSUM: banks 0-3 h1 slots; 4,5 h2; 6,7 scoresT; ui reuses 4-7 rows {32j}.
ui: attn-col stationary (M=1), col-group packed 4 ways.
Out: (256,128) f32.
"""

import numpy as np
import ml_dtypes

BF16 = ml_dtypes.bfloat16

E = 128
H = 64
B = 2048
L = 200
NCORES = 8
BL = B // NCORES          # 256
NBLK = 4
BB = BL // NBLK           # 64
NPAIR = BB // 2           # 32
L0 = 128
L1 = L - L0               # 72

_NC_CACHE = {}


class Sem:
    def __init__(self, handle):
        self.h = handle
        self.val = 0

    def inc(self, instr, n=1):
        instr.then_inc(self.h, n)
        self.val += n
        return self.val


def build_nc():
    import concourse.bass as bass
    import concourse.mybir as mybir
    from contextlib import ExitStack

    dt = mybir.dt
    AF = mybir.ActivationFunctionType
    AO = mybir.AluOpType

    nc = bass.Bass("TRN2", target_bir_lowering=False)

    d_keysT = nc.declare_dram_parameter("keysT", [E, BL * L], dt.bfloat16, False)
    d_nat0 = nc.declare_dram_parameter("nat0", [L0, BL * E], dt.bfloat16, False)
    d_nat1 = nc.declare_dram_parameter("nat1", [L1, BL * E], dt.bfloat16, False)
    d_wall = nc.declare_dram_parameter("wall", [E, NBLK * H * BB], dt.bfloat16, False)
    d_qub = nc.declare_dram_parameter("qub", [2 * H, BL // 2], dt.float32, False)
    d_mT0 = nc.declare_dram_parameter("maskT0", [L0, BL], dt.bfloat16, False)
    d_mT1 = nc.declare_dram_parameter("maskT1", [L1, BL], dt.bfloat16, False)
    d_b2 = nc.declare_dram_parameter("b2stk", [2 * H, 1], dt.float32, False)
    d_W2 = nc.declare_dram_parameter("W2blk", [2 * H, 2 * H], dt.bfloat16, False)
    d_W3 = nc.declare_dram_parameter("W3blk", [2 * H, 2], dt.bfloat16, False)
    d_out = nc.declare_dram_parameter("out", [BL, E], dt.float32, True)

    es = ExitStack()
    sb = lambda n, s, d: es.enter_context(nc.sbuf_tensor(n, s, d))

    s_keysT = [sb(f"s_keysT{i}", [E, BB * L], dt.bfloat16) for i in range(2)]
    s_nat0 = [sb(f"s_nat0{i}", [L0, BB * E], dt.bfloat16) for i in range(2)]
    s_nat1 = [sb(f"s_nat1{i}", [L1, BB * E], dt.bfloat16) for i in range(2)]
    s_wall = sb("s_wall", [E, NBLK * H * BB], dt.bfloat16)
    s_qub = sb("s_qub", [2 * H, BL // 2], dt.float32)
    s_mT0 = sb("s_mT0", [L0, BL], dt.bfloat16)
    s_mT1 = sb("s_mT1", [L1, BL], dt.bfloat16)
    s_b2 = sb("s_b2", [2 * H, 1], dt.float32)
    s_W2 = sb("s_W2", [2 * H, 2 * H], dt.bfloat16)
    s_W3 = sb("s_W3", [2 * H, 2], dt.bfloat16)
    s_h1r = sb("s_h1r", [2 * H, NPAIR * L], dt.bfloat16)
    s_h2r = sb("s_h2r", [2 * H, NPAIR * L], dt.bfloat16)
    s_exp0 = sb("s_exp0", [L0, BB], dt.bfloat16)
    s_exp1 = sb("s_exp1", [L1, BB], dt.bfloat16)
    s_att0 = sb("s_att0", [L0, BB], dt.bfloat16)
    s_att1 = sb("s_att1", [L1, BB], dt.bfloat16)
    s_rcp = sb("s_rcp", [1, BB], dt.float32)
    s_att0n = sb("s_att0n", [L0, BB], dt.bfloat16)
    s_att1n = sb("s_att1n", [L1, BB], dt.bfloat16)
    s_ones = sb("s_ones", [128, 1], dt.bfloat16)
    s_onesr = sb("s_onesr", [1, 128], dt.float32)
    s_warm = sb("s_warm", [128, 512], dt.bfloat16)
    s_uiA = [sb(f"s_uiA{i}", [97, 1024], dt.float32) for i in range(2)]
    s_uiB = [sb(f"s_uiB{i}", [97, 1024], dt.float32) for i in range(2)]

    ps = es.enter_context(nc.psum_tensor("ps", [128, 8, 512], dt.float32))
    ps_h1 = lambda slot: ps[:, slot, 0:L]                # banks 0..3
    ps_h2 = lambda slot: ps[:, 4 + slot, 0:2 * L]        # banks 4..6 (3 slots)
    ps_sc0 = ps[0:L0, 7, 0:BB]
    ps_sc1 = ps[0:L1, 7, BB:2 * BB]
    ps_den = ps[0:1, 1, 0:BB]                            # bank 1 (post-h1)
    ps_bc0 = ps[0:L0, 2, 0:BB]                           # bank 2
    ps_bc1 = ps[0:L1, 3, 0:BB]                           # bank 3

    # ui slot for b in [0,64): partition 32*(b//16), bank 4 + (b%16)//4,
    # offset 128*(b%4). Row 32j holds b = 16j..16j+16 (contiguous out rows).
    def ps_ui(b):
        j = b // 16
        q = b % 16
        return ps[32 * j:32 * j + 1, 4 + q // 4,
                  128 * (q % 4):128 * (q % 4) + 128]

    N_SMALL = 6
    THR_SMALL = N_SMALL * 16

    sems = {n: es.enter_context(nc.semaphore(n)) for n in [
        "m_dsm", "m_bK0", "m_bK1", "m_bN0", "m_bN1", "m_bK0b", "m_bN0b",
        "m_bN1b", "m_dui0", "m_dui1",
        "m_w0", "m_w1", "m_w2", "m_w3",
        "m_h1", "m_r1a", "m_r1v", "m_h2", "m_r2a", "m_r2v", "m_sc", "m_exp",
        "m_msk", "m_den", "m_rcp", "m_bc", "m_att", "m_ui", "m_cpA", "m_cpB",
        "m_ms0"]}
    if True:
        dsm = Sem(sems["m_dsm"])
        bK = [Sem(sems["m_bK0"]), Sem(sems["m_bK1"])]
        bN = [Sem(sems["m_bN0"]), Sem(sems["m_bN1"])]
        bK0b = Sem(sems["m_bK0b"])
        bNb = [Sem(sems["m_bN0b"]), Sem(sems["m_bN1b"])]
        dui = [Sem(sems["m_dui0"]), Sem(sems["m_dui1"])]
        wl = [Sem(sems[f"m_w{i}"]) for i in range(4)]
        h1s = Sem(sems["m_h1"])
        r1 = [Sem(sems["m_r1a"]), Sem(sems["m_r1v"])]   # even pairs ACT, odd DVE
        h2s = Sem(sems["m_h2"])
        r2 = [Sem(sems["m_r2a"]), Sem(sems["m_r2v"])]   # even pps ACT, odd DVE
        scs = Sem(sems["m_sc"])
        exps = Sem(sems["m_exp"])
        msks = Sem(sems["m_msk"])
        dens = Sem(sems["m_den"])
        rcps = Sem(sems["m_rcp"])
        bcs = Sem(sems["m_bc"])
        atts = Sem(sems["m_att"])
        uis = Sem(sems["m_ui"])
        cpA = Sem(sems["m_cpA"])
        cpB = Sem(sems["m_cpB"])
        ms0 = Sem(sems["m_ms0"])

        # relu1 of (k,p): parity p%2 (0=ACT,1=DVE), count 16k + p//2 + 1
        r1cnt = lambda k, p: 16 * k + p // 2 + 1
        # relu2 of (k,pp): parity pp%2, count 8k + pp//2 + 1
        r2cnt = lambda k, pp: 8 * k + pp // 2 + 1

        with nc.Block() as block:

            # -------- GPSIMD: all DMAs --------
            @block.gpsimd
            def _(g):
                bK[0].inc(g.dma_start(
                    out=s_keysT[0][:, 0:BB * L // 2],
                    in_=d_keysT[:, 0:BB * L // 2]), 16)
                for dst, src in [
                    (s_mT0, d_mT0), (s_mT1, d_mT1), (s_qub, d_qub),
                    (s_b2, d_b2), (s_W2, d_W2), (s_W3, d_W3),
                ]:
                    dsm.inc(g.dma_start(out=dst[:, :], in_=src[:, :]), 16)
                bK[1].inc(g.dma_start(
                    out=s_keysT[1][:, :],
                    in_=d_keysT[:, BB * L:2 * BB * L]), 16)
                for k in range(2):
                    buf = k % 2
                    bN[buf].inc(g.dma_start(
                        out=s_nat0[buf][:, :],
                        in_=d_nat0[:, k * BB * E:(k + 1) * BB * E]), 16)
                # interleaved: block k+2 inputs + ui out-DMA of block k
                for k in range(NBLK):
                    g.wait_ge(uis.h, k + 1)
                    kk = k + 2
                    if kk < NBLK:
                        buf = kk % 2
                        bK[buf].inc(g.dma_start(
                            out=s_keysT[buf][:, :],
                            in_=d_keysT[:, kk * BB * L:(kk + 1) * BB * L]), 16)
                        bN[buf].inc(g.dma_start(
                            out=s_nat0[buf][:, :],
                            in_=d_nat0[:, kk * BB * E:(kk + 1) * BB * E]), 16)
                        bN[buf].inc(g.dma_start(
                            out=s_nat1[buf][:, :],
                            in_=d_nat1[:, kk * BB * E:(kk + 1) * BB * E]), 16)
                    g.wait_ge(cpA.h, k + 1)
                    g.wait_ge(cpB.h, k + 1)
                    for j in range(4):
                        dui[k % 2].inc(g.dma_start(
                            out=d_out[k * BB + 16 * j:k * BB + 16 * j + 8, :],
                            in_=s_uiA[k % 2][32 * j:32 * j + 1, :]), 16)
                        dui[k % 2].inc(g.dma_start(
                            out=d_out[k * BB + 16 * j + 8:k * BB + 16 * j + 16, :],
                            in_=s_uiB[k % 2][32 * j:32 * j + 1, :]), 16)

            # -------- SYNC: second DMA ring (HWDGE) --------
            @block.sync
            def _(sy):
                bK0b.inc(sy.dma_start(
                    out=s_keysT[0][:, BB * L // 2:],
                    in_=d_keysT[:, BB * L // 2:BB * L]), 16)
                for k in range(NBLK):
                    wl[k].inc(sy.dma_start(
                        out=s_wall[:, k * H * BB:(k + 1) * H * BB],
                        in_=d_wall[:, k * H * BB:(k + 1) * H * BB]), 16)
                for k in range(2):
                    buf = k % 2
                    bNb[buf].inc(sy.dma_start(
                        out=s_nat1[buf][:, :],
                        in_=d_nat1[:, k * BB * E:(k + 1) * BB * E]), 16)

            # ---- DVE: memsets; relu1 odd / relu2 odd; softmax; cpB ----
            @block.vector
            def _(v):
                v.memset(s_ones[:, :], 1.0)
                v.memset(s_onesr[:, :], 1.0)
                v.memset(s_warm[:, :], 0.001)
                ins = v.memset(ps[:, 0:8, 0:512], 0.0)
                ms0.inc(ins)
                v.wait_ge(dsm.h, THR_SMALL)

                def emit_cpB(kk):
                    v.wait_ge(uis.h, kk + 1)
                    if kk >= 2:
                        v.wait_ge(dui[kk % 2].h, 128 * ((kk - 2) // 2 + 1))
                    ins = v.tensor_copy(out=s_uiB[kk % 2][:, :],
                                        in_=ps[0:97, 6:8, 0:512])
                    cpB.inc(ins)

                for k in range(NBLK):
                    for p in range(1, NPAIR, 2):      # odd pairs relu1
                        if k > 0 and p == 17:
                            emit_cpB(k - 1)
                        v.wait_ge(h1s.h, 32 * k + p + 1)
                        ins = v.tensor_scalar(
                            out=s_h1r[:, p * L:(p + 1) * L],
                            in0=ps_h1(p % 4)[:, :],
                            scalar1=s_qub[:, k * NPAIR + p:k * NPAIR + p + 1],
                            scalar2=0.0, op0=AO.add, op1=AO.max)
                        r1[1].inc(ins)
                    for pp in range(1, NPAIR // 2, 2):  # odd pps relu2
                        v.wait_ge(h2s.h, 16 * k + pp + 1)
                        ins = v.tensor_scalar(
                            out=s_h2r[:, 2 * pp * L:(2 * pp + 2) * L],
                            in0=ps_h2(pp % 3)[:, :],
                            scalar1=s_b2[:, 0:1], scalar2=0.0,
                            op0=AO.add, op1=AO.max)
                        r2[1].inc(ins)
                    # p = exp * mask
                    v.wait_ge(exps.h, 2 * k + 2)
                    v.tensor_tensor(
                        out=s_att0[:, :], in0=s_exp0[:, :],
                        in1=s_mT0[:, k * BB:(k + 1) * BB], op=AO.mult)
                    ins = v.tensor_tensor(
                        out=s_att1[:, :], in0=s_exp1[:, :],
                        in1=s_mT1[:, k * BB:(k + 1) * BB], op=AO.mult)
                    msks.inc(ins)
                    v.wait_ge(dens.h, k + 1)
                    ins = v.reciprocal(out=s_rcp[:, :], in_=ps_den)
                    rcps.inc(ins)
                    v.wait_ge(bcs.h, k + 1)
                    v.tensor_tensor(out=s_att0n[:, :], in0=s_att0[:, :],
                                    in1=ps_bc0, op=AO.mult)
                    ins = v.tensor_tensor(out=s_att1n[:, :], in0=s_att1[:, :],
                                          in1=ps_bc1, op=AO.mult)
                    atts.inc(ins)
                emit_cpB(NBLK - 1)

            # -------- PE (software-pipelined) --------
            @block.tensor
            def _(t):
                def emit_ui(kk, i0, i1):
                    # ui mms for block kk (data buf kk%2), i in [i0,i1) x 4 col
                    # groups: b = 16j + i cycles groups for 4-way overlap
                    bufu = kk % 2
                    last = None
                    for b in [16 * j + i for i in range(i0, i1)
                              for j in range(4)]:
                        tp = (0, 32 * (b // 16))
                        t.matmul(ps_ui(b),
                                 lhsT=s_att0n[:, b:b + 1],
                                 rhs=s_nat0[bufu][:, b * E:(b + 1) * E],
                                 start=True, stop=False, tile_position=tp)
                        last = t.matmul(
                            ps_ui(b),
                            lhsT=s_att1n[:, b:b + 1],
                            rhs=s_nat1[bufu][:, b * E:(b + 1) * E],
                            start=False, stop=True, tile_position=tp)
                    return last

                def emit_h2(k, pp):
                    if k > 0 and pp == 0:
                        t.wait_ge(cpA.h, k)
                        t.wait_ge(cpB.h, k)
                    t.wait_ge(r1[0].h, 16 * k + pp + 1)
                    t.wait_ge(r1[1].h, 16 * k + pp + 1)
                    if pp >= 3:
                        t.wait_ge(r2[(pp - 3) % 2].h, r2cnt(k, pp - 3))
                    ins = t.matmul(
                        ps_h2(pp % 3)[:, :],
                        lhsT=s_W2[:, :],
                        rhs=s_h1r[:, 2 * pp * L:(2 * pp + 2) * L],
                        start=True, stop=True)
                    h2s.inc(ins)

                t.wait_ge(ms0.h, 1)
                for _ in range(24):   # HAM warm-up during initial DMA wait
                    t.matmul(ps[0:1, 0, 0:512], lhsT=s_ones[:, :],
                             rhs=s_warm[:, :], start=True, stop=True)
                for k in range(NBLK):
                    buf = k % 2
                    t.wait_ge(bK[buf].h,
                              {0: 16, 1: 16, 2: 32, 3: 32}[k])
                    if k == 0:
                        t.wait_ge(bK0b.h, 16)
                    t.wait_ge(wl[k].h, 16)
                    for p in range(NPAIR):
                        pk, pq = (k, p - 4) if p >= 4 else (k - 1, p + 28)
                        if pk >= 0:
                            t.wait_ge(r1[pq % 2].h, r1cnt(pk, pq))
                        for j in range(2):
                            b = 2 * p + j
                            gb = k * H * BB + b
                            ins = t.matmul(
                                ps_h1(p % 4)[j * H:(j + 1) * H, :],
                                lhsT=s_wall[:, gb:(k + 1) * H * BB:BB],
                                rhs=s_keysT[buf][:, b * L:(b + 1) * L],
                                start=True, stop=True)
                        h1s.inc(ins)
                    for pp in range(NPAIR // 2):
                        emit_h2(k, pp)
                    # --- scores ---
                    if k > 0:
                        t.wait_ge(exps.h, 2 * k)
                    for p in range(NPAIR):
                        t.wait_ge(r2[(p // 2) % 2].h, r2cnt(k, p // 2))
                        t.matmul(ps_sc0[:, 2 * p:2 * p + 2],
                                 lhsT=s_h2r[:, p * L:p * L + L0],
                                 rhs=s_W3[:, :], start=True, stop=True)
                        ins = t.matmul(ps_sc1[:, 2 * p:2 * p + 2],
                                       lhsT=s_h2r[:, p * L + L0:(p + 1) * L],
                                       rhs=s_W3[:, :], start=True, stop=True)
                    scs.inc(ins)
                    # --- denom ---
                    t.wait_ge(msks.h, k + 1)
                    t.matmul(ps_den, lhsT=s_ones[:, :], rhs=s_att0[:, :],
                             start=True, stop=False)
                    ins = t.matmul(ps_den, lhsT=s_ones[0:L1, :],
                                   rhs=s_att1[:, :], start=False, stop=True)
                    dens.inc(ins)
                    # --- bcast 1/denom ---
                    t.wait_ge(rcps.h, k + 1)
                    t.matmul(ps_bc0, lhsT=s_onesr[:, 0:L0], rhs=s_rcp[:, :],
                             start=True, stop=True)
                    ins = t.matmul(ps_bc1, lhsT=s_onesr[:, 0:L1],
                                   rhs=s_rcp[:, :], start=True, stop=True)
                    bcs.inc(ins)
                    # --- ui (banks 4..7) ---
                    t.wait_ge(atts.h, k + 1)
                    t.wait_ge(bN[buf].h, {0: 16, 1: 16, 2: 48, 3: 48}[k])
                    if k < 2:
                        t.wait_ge(bNb[buf].h, 16)
                    ins = emit_ui(k, 0, 16)
                    uis.inc(ins)

            # -------- ACT: relu1 even / relu2 even; exp; cpA --------
            @block.scalar
            def _(a):
                a.wait_ge(dsm.h, THR_SMALL)

                def emit_cpA(kk):
                    a.wait_ge(uis.h, kk + 1)
                    if kk >= 2:
                        a.wait_ge(dui[kk % 2].h, 128 * ((kk - 2) // 2 + 1))
                    ins = a.activation(out=s_uiA[kk % 2][:, :],
                                       in_=ps[0:97, 4:6, 0:512],
                                       func=AF.Copy, bias=0.0, scale=1.0)
                    cpA.inc(ins)

                for k in range(NBLK):
                    for p in range(0, NPAIR, 2):      # even pairs relu1
                        if k > 0 and p == 16:
                            emit_cpA(k - 1)
                        a.wait_ge(h1s.h, 32 * k + p + 1)
                        ins = a.activation(
                            out=s_h1r[:, p * L:(p + 1) * L],
                            in_=ps_h1(p % 4)[:, :],
                            func=AF.Relu,
                            bias=s_qub[:, k * NPAIR + p:k * NPAIR + p + 1],
                            scale=1.0)
                        r1[0].inc(ins)
                    for pp in range(0, NPAIR // 2, 2):  # even pps relu2
                        a.wait_ge(h2s.h, 16 * k + pp + 1)
                        ins = a.activation(
                            out=s_h2r[:, 2 * pp * L:(2 * pp + 2) * L],
                            in_=ps_h2(pp % 3)[:, :],
                            func=AF.Relu, bias=s_b2[:, 0:1], scale=1.0)
                        r2[0].inc(ins)
                    a.wait_ge(scs.h, k + 1)
                    if k > 0:
                        a.wait_ge(msks.h, k)
                    ins = a.activation(out=s_exp0[:, :], in_=ps_sc0,
                                       func=AF.Exp, bias=0.0, scale=1.0)
                    exps.inc(ins)
                    ins = a.activation(out=s_exp1[:, :], in_=ps_sc1,
                                       func=AF.Exp, bias=0.0, scale=1.0)
                    exps.inc(ins)
                emit_cpA(NBLK - 1)

    es.close()
    return nc


def _prep_core(inputs, c):
    q = np.asarray(inputs["query"][c * BL:(c + 1) * BL], np.float32)
    keys = np.asarray(inputs["keys"][c * BL:(c + 1) * BL], np.float32)
    mask = np.asarray(inputs["mask"][c * BL:(c + 1) * BL])
    W1 = np.asarray(inputs["W1"], np.float32)
    U = W1[0:E] + W1[3 * E:4 * E]
    V = W1[E:2 * E] - W1[3 * E:4 * E]
    C = W1[2 * E:3 * E]
    W2 = np.asarray(inputs["W2"], np.float32)
    W3 = np.asarray(inputs["W3"], np.float32)
    b1 = np.asarray(inputs["b1"], np.float32)
    b2 = np.asarray(inputs["b2"], np.float32)

    keysT = np.ascontiguousarray(
        keys.transpose(2, 0, 1).reshape(E, BL * L)).astype(BF16)
    nat0 = np.ascontiguousarray(
        keys[:, 0:L0, :].transpose(1, 0, 2).reshape(L0, BL * E)).astype(BF16)
    nat1 = np.ascontiguousarray(
        keys[:, L0:L, :].transpose(1, 0, 2).reshape(L1, BL * E)).astype(BF16)
    mT = np.ascontiguousarray(mask.T.astype(np.float32))

    # W_all[e, blk, h, b_local] = V[e,h] + q[b,e]*C[e,h]
    wall = V[:, None, :] + q.T[:, :, None] * C[:, None, :]    # (E, BL, H)
    wall = wall.reshape(E, NBLK, BB, H).transpose(0, 1, 3, 2)  # (E, blk, H, b)
    wall = np.ascontiguousarray(wall.reshape(E, NBLK * H * BB)).astype(BF16)

    # qUb stacked per pair: [even-b (64); odd-b (64)] x 128 pairs, f32
    qu = q @ U + b1[None, :]                                  # (BL, H)
    qub = np.empty((2 * H, BL // 2), np.float32)
    qub[0:H] = qu[0::2].T
    qub[H:] = qu[1::2].T

    W2blk = np.zeros((2 * H, 2 * H), np.float32)
    W2blk[0:H, 0:H] = W2
    W2blk[H:, H:] = W2
    W3blk = np.zeros((2 * H, 2), np.float32)
    W3blk[0:H, 0] = W3[:, 0]
    W3blk[H:, 1] = W3[:, 0]
    b2stk = np.concatenate([b2, b2]).reshape(2 * H, 1).astype(np.float32)
    return {
        "keysT": keysT, "nat0": nat0, "nat1": nat1,
        "wall": wall, "qub": qub,
        "maskT0": mT[0:L0].astype(BF16), "maskT1": mT[L0:L].astype(BF16),
        "b2stk": b2stk,
        "W2blk": W2blk.astype(BF16), "W3blk": W3blk.astype(BF16),
    }


def kernel(**inputs):
    from concourse.bass_utils import run_bass_kernel_spmd

    if "nc" not in _NC_CACHE:
        _NC_CACHE["nc"] = build_nc()
    nc = _NC_CACHE["nc"]

    in_maps = [_prep_core(inputs, c) for c in range(NCORES)]
    res = run_bass_kernel_spmd(nc, in_maps, core_ids=list(range(NCORES)))
    out = np.concatenate([np.asarray(r["out"], np.float32)
                          for r in res.results], axis=0)

    mask = np.asarray(inputs["mask"])
    all_pad = mask.sum(axis=1) == 0
    if all_pad.any():
        out = np.where(all_pad[:, None],
                       np.asarray(inputs["no_hist"], np.float32)[None, :], out)
    return out.astype(np.float32)

